# revision 40
# baseline (speedup 1.0000x reference)
"""Trainium2 (Bass/Tile) segment-sum kernel, 8-core SPMD.

Computes out[v, :] = sum over rows n with X_node[n] == v of H[n, :]
(equivalent to jax.ops.segment_sum(H, X_node, num_segments=V)).

Strategy:
  host: stable-argsort rows by segment id; split the sorted order into 8
    contiguous chunks (one per NeuronCore) so each core covers a narrow,
    contiguous segment range (~V/8 segments). Within a core, rows are
    greedily grouped into W windows, each covering <=S consecutive
    segments and <=T*128 rows; each window is laid out as T tiles of 128
    rows, padded with dummy rows (lid=255) so all 8 cores run ONE static
    SPMD program.
  precision (fp8 modes, default): H is quantized to fp8-e4m3 with
    per-(core, segment) ERROR FEEDBACK (noise shaping): rows of a segment
    are quantized in sorted order, each carrying the accumulated
    quantization residual of its predecessors. The device sums the fp8
    codes exactly (matmul into f32 PSUM), so the error of each segment
    sum telescopes to just the final dropped carry -- ONE fp8 rounding
    (~0.027 abs) against a sum of magnitude ~sqrt(32), i.e. ~5e-3
    relative, at HALF the HBM bytes of bf16.
  device (mode "fp8", S=32): per tile, ONE matmul with the H tile as the
    128-col STATIONARY operand (fp8 FWL weight load, ~27 ns) and the
    tile's [128, 32] one-hot as the narrow MOVING operand (~25 ns):
    PSUM[d, v] += H_tile^T @ onehot. GROUP=8 consecutive windows share
    one [128, 8*32] PSUM bank at different free-dim offsets, evacuated
    (and DMA'd out) once per group; one group = one ~0.9 MiB h DMA.
    The narrow windows also shrink the DVE one-hot build 4x vs a
    128-seg window (the old bottleneck: DVE is_equal runs at 1x,
    ~112 G elem/s -- broadcast APs and 1-byte outputs disqualify the
    2x/4x perf modes).
  host: transpose-add the per-core [G, 128d, 8*32v] group strips into
    the full [V, D] output (windows of adjacent cores may overlap;
    addition is exact).

Measured on the target data: ~100 us HW exec across 8 cores (vs ~292 us
for the 2-plane bf16 predecessor), relative error 5.0e-3 vs the f32
reference (gate: 2e-2). Per-core HBM traffic is ~28 MB (25.7 MB H fp8 +
lid + bf16 out) streaming at the measured ~315-330 GB/s under full
8-core load, plus ~9 us NEFF preamble and ~8 us pipeline-drain tail --
i.e. the kernel sits at the fp8 memory roofline; engines peak at
DVE ~57 us, PE ~48 us, ACT ~20 us, all hidden under the DMA stream.

Fallback modes via SEGSUM_MODE: "fp8w128" (one-hot stationary, 128-seg
windows, ~239 us), "bf16" (single plane), "bf16x2" (hi+lo, ~2.5e-6).
"""

import os

import numpy as np
from contextlib import ExitStack

import ml_dtypes
import concourse.bass as bass
import concourse.tile as tile
from concourse import bacc, mybir
from concourse.bass_utils import run_bass_kernel_spmd

F32 = mybir.dt.float32
BF16 = mybir.dt.bfloat16
FP8 = mybir.dt.float8e4
NP_BF16 = ml_dtypes.bfloat16
NP_FP8 = ml_dtypes.float8_e4m3
P = 128  # partitions / tile rows
D = 128  # feature dim
N_CORES = 8
PAD_LID = 255.0

LAST_RESULTS = None  # test-harness hook: BassKernelResults of the last run
_NC_CACHE = {}


def _mode():
    return os.environ.get("SEGSUM_MODE", "fp8")


# ---------------------------------------------------------------------------
# mode "fp8": span-32 windows, H-stationary matmuls, grouped PSUM
# ---------------------------------------------------------------------------
S_NARROW = 32  # segments per window
GROUP = 8  # windows per PSUM bank / output strip / h DMA chunk
T_NARROW_CANDIDATES = (7, 8, 9, 10, 11)


CHUNK = 1  # groups per h DMA (8 windows x T tiles ~ 0.9 MiB per DMA)


def _build_nc_narrow(W: int, T: int):
    G = W // GROUP
    C = G // CHUNK
    nc = bacc.Bacc(
        "TRN2",
        target_bir_lowering=False,
        debug=False,
        enable_asserts=False,
        num_devices=N_CORES,
    )
    # h[c, p, (g, j, t, d)] -- per-partition contiguous CHUNK*8*T*D-byte runs
    U8 = mybir.dt.uint8
    h = nc.dram_tensor("h", [C, P, CHUNK * GROUP * T * D], FP8, kind="ExternalInput")
    lid = nc.dram_tensor("lid", [P, W * T], U8, kind="ExternalInput")
    iota = nc.dram_tensor("iota", [P, S_NARROW], U8, kind="ExternalInput")
    out = nc.dram_tensor("out", [G, P, GROUP * S_NARROW], BF16, kind="ExternalOutput")

    with tile.TileContext(nc) as tc, ExitStack() as ctx:
        const = ctx.enter_context(tc.tile_pool(name="const", bufs=1))
        hpool = ctx.enter_context(tc.tile_pool(name="hw", bufs=6))
        ohpool = ctx.enter_context(tc.tile_pool(name="oh", bufs=4))
        opool = ctx.enter_context(tc.tile_pool(name="ot", bufs=6))
        psum = ctx.enter_context(tc.tile_pool(name="acc", bufs=4, space="PSUM"))

        def load_h(c):
            ht = hpool.tile([P, CHUNK * GROUP * T * D], FP8, tag="ht")
            nc.sync.dma_start(ht[:], h[c])
            return ht

        HHALF = CHUNK * GROUP * T * D // 2

        def load_h_half(c, half):
            # the final chunk loads in halves so its first windows' matmuls
            # overlap the second half's transfer, shortening the kernel tail
            ht = hpool.tile([P, HHALF], FP8, tag="hth")
            nc.sync.dma_start(ht[:], h[c][:, half * HHALF : (half + 1) * HHALF])
            return ht

        # issue the first chunks' loads before the constants so the SDMA
        # engines have bulk work immediately
        hts = {c: load_h(c) for c in range(min(3, C))}

        # constants go via the ACT ring so they land immediately instead of
        # queueing behind the hoisted bulk loads on the SP ring
        iota_sb = const.tile([P, S_NARROW], U8)
        nc.scalar.dma_start(iota_sb[:], iota[:])
        lid_sb = const.tile([P, W * T], U8)
        nc.scalar.dma_start(lid_sb[:], lid[:])

        for g in range(G):
            c = g // CHUNK
            last = g == G - 1
            if last:
                hhs = [load_h_half(c, 0), load_h_half(c, 1)]
            elif c not in hts:
                hts[c] = load_h(c)
            if not last:
                ht = hts[c]
            hoff = (g % CHUNK) * GROUP * T * D
            # one fused DVE op builds the whole group's one-hot tiles:
            # oh[p, (j, t), v] = (iota[p, v] == lid[p, (8g+j)*T + t])
            oh = ohpool.tile([P, GROUP * T, S_NARROW], FP8)
            nc.vector.tensor_tensor(
                oh[:],
                iota_sb[:].unsqueeze(1).broadcast_to((P, GROUP * T, S_NARROW)),
                lid_sb[:, GROUP * T * g : GROUP * T * (g + 1)]
                .unsqueeze(2)
                .broadcast_to((P, GROUP * T, S_NARROW)),
                mybir.AluOpType.is_equal,
            )
            # 8 windows accumulate into disjoint 32-col slices of one bank
            acc = psum.tile([P, GROUP * S_NARROW], F32)
            for j in range(GROUP):
                for t in range(T):
                    jt = j * T + t
                    if last:
                        half = (hoff + jt * D) // HHALF
                        loc = hoff + jt * D - half * HHALF
                        src = hhs[half][:, loc : loc + D]
                    else:
                        src = ht[:, hoff + jt * D : hoff + (jt + 1) * D]
                    nc.tensor.matmul(
                        acc[:, j * S_NARROW : (j + 1) * S_NARROW],
                        src,
                        oh[:, jt, :],
                        start=(t == 0),
                        stop=(t == T - 1),
                    )
            # DVE evacuation (ACT copies are ~9x slower) with bf16 downcast
            ot = opool.tile([P, GROUP * S_NARROW], BF16)
            nc.vector.tensor_copy(ot[:], acc[:])
            nc.scalar.dma_start(out[g], ot[:])

    nc.compile()
    return nc


def _quantize_fp8_feedback(Hs: np.ndarray, sidx: np.ndarray, nloc: int):
    """Error-feedback fp8 quantization of the sorted rows Hs.

    Rows are grouped by (core, segment); within each group the quantization
    residual is carried into the next row, so the group's exact fp8 sum
    differs from the f32 sum only by the final dropped carry.
    """
    N = Hs.shape[0]
    new = np.empty(N, bool)
    new[0] = True
    new[1:] = sidx[1:] != sidx[:-1]
    new[nloc * np.arange(1, N_CORES)] = True  # feedback must not cross cores
    gid = np.cumsum(new) - 1
    gstart = np.flatnonzero(new)
    gsize = np.diff(np.append(gstart, N))

    Q = np.empty(Hs.shape, NP_FP8)
    c = np.zeros((len(gstart), Hs.shape[1]), np.float32)
    for r in range(int(gsize.max())):
        g = np.flatnonzero(gsize > r)
        rows = gstart[g] + r
        t = Hs[rows] + c[g]
        q = t.astype(NP_FP8)
        c[g] = t - q.astype(np.float32)
        Q[rows] = q
    return Q


def _sort_and_window(X: np.ndarray, V: int, span: int, t_candidates):
    """Stable sort + per-core greedy windowing (span-limited, row-capped).

    Returns (perm, sidx, bounds, T, W) where W is padded to GROUP align.
    """
    N = len(X)
    nloc = N // N_CORES
    perm = np.argsort(X, kind="stable")
    sidx = X[perm]

    def greedy(T):
        cap = T * P
        bounds = []  # per core: row-rank boundaries [0, ..., nloc]
        for k in range(N_CORES):
            s = sidx[k * nloc : (k + 1) * nloc]
            b = [0]
            r = 0
            while r < nloc:
                r = min(r + cap, int(np.searchsorted(s, s[r] + span, side="left")))
                b.append(r)
            bounds.append(np.asarray(b, np.int64))
        return bounds, max(len(b) - 1 for b in bounds)

    best = None
    for T in t_candidates:
        bounds, W = greedy(T)
        if best is None or W * T < best[2] * best[1]:
            best = (bounds, T, W)
    bounds, T, W = best
    return perm, sidx, bounds, T, W


def _row_windows(sidx, bounds, W, V, nloc):
    """Per-row window index / rank; per-window base segment."""
    N = len(sidx)
    wbase = np.full((N_CORES, W), V, np.int64)  # pad windows point past V
    win = np.empty(N, np.int64)
    rank = np.empty(N, np.int64)
    for k in range(N_CORES):
        b = bounds[k]
        s = sidx[k * nloc : (k + 1) * nloc]
        idx = np.arange(nloc)
        wk = np.searchsorted(b, idx, side="right") - 1
        win[k * nloc : (k + 1) * nloc] = wk
        rank[k * nloc : (k + 1) * nloc] = idx - b[wk]
        wbase[k, : len(b) - 1] = s[b[:-1]]
    return wbase, win, rank


def _prepare_narrow(H: np.ndarray, X: np.ndarray, V: int):
    N, Dd = H.shape
    assert Dd == D and N % N_CORES == 0
    nloc = N // N_CORES
    X = np.ascontiguousarray(X).astype(np.int64, copy=False)
    perm, sidx, bounds, T, W = _sort_and_window(
        X, V, S_NARROW, T_NARROW_CANDIDATES
    )
    W = -(-W // (GROUP * CHUNK)) * (GROUP * CHUNK)  # pad to DMA-chunk multiple
    G = W // GROUP
    cap = T * P

    wbase, win, rank = _row_windows(sidx, bounds, W, V, nloc)
    k_arr = np.repeat(np.arange(N_CORES), nloc)
    lid_val = sidx - wbase[k_arr, win]
    # slot layout: [core][window][partition][tile] so each partition's DRAM
    # run within a group is contiguous
    slot = (k_arr * W + win) * cap + (rank & (P - 1)) * T + (rank >> 7)

    total = N_CORES * W * cap
    Qs = _quantize_fp8_feedback(H[perm], sidx, nloc)
    Hp = np.zeros((total, D), NP_FP8)
    Hp[slot] = Qs
    # [(k, w, p, t), d] -> [k, c, p, (g, j, t, d)]
    C = G // CHUNK
    Hp = (
        Hp.reshape(N_CORES, C, CHUNK * GROUP, P, T * D)
        .transpose(0, 1, 3, 2, 4)
        .reshape(N_CORES, C, P, CHUNK * GROUP * T * D)
    )
    Hp = np.ascontiguousarray(Hp)

    lid = np.full(total, 255, np.uint8)
    lid[slot] = lid_val.astype(np.uint8)
    lid = (
        lid.reshape(N_CORES, W, P, T).transpose(0, 2, 1, 3).reshape(N_CORES, P, W * T)
    )
    lid = np.ascontiguousarray(lid)

    iota = np.ascontiguousarray(
        np.broadcast_to(np.arange(S_NARROW, dtype=np.uint8), (P, S_NARROW))
    )

    in_maps = [{"h": Hp[k], "lid": lid[k], "iota": iota} for k in range(N_CORES)]
    return in_maps, wbase, W, T


def _kernel_narrow(H, X, V, trace):
    global LAST_RESULTS
    in_maps, wbase, W, T = _prepare_narrow(H, X, V)
    key = ("fp8", W, T)
    if key not in _NC_CACHE:
        _NC_CACHE[key] = _build_nc_narrow(W, T)
    nc = _NC_CACHE[key]
    res = run_bass_kernel_spmd(nc, in_maps, list(range(N_CORES)), trace=trace)
    LAST_RESULTS = res

    out = np.zeros((V + S_NARROW, D), np.float32)
    for k in range(N_CORES):
        o = np.asarray(res.results[k]["out"]).astype(np.float32)  # [G, d, 16*16v]
        for w in range(W):
            b = int(wbase[k, w])
            g, j = divmod(w, GROUP)
            out[b : b + S_NARROW] += o[g][:, j * S_NARROW : (j + 1) * S_NARROW].T
    return np.ascontiguousarray(out[:V])


# ---------------------------------------------------------------------------
# legacy modes: 128-seg windows, one-hot stationary
# ---------------------------------------------------------------------------
T_WIDE_CANDIDATES = (28, 29, 30, 31, 32)


def _build_nc_wide(W: int, T: int, mode: str):
    planes = 2 if mode == "bf16x2" else 1
    hdt = FP8 if mode == "fp8w128" else BF16

    nc = bacc.Bacc(
        "TRN2",
        target_bir_lowering=False,
        debug=False,
        enable_asserts=False,
        num_devices=N_CORES,
    )
    h = nc.dram_tensor("h", [W, P, T * planes * D], hdt, kind="ExternalInput")
    lid = nc.dram_tensor("lid", [P, W * T], BF16, kind="ExternalInput")
    iota = nc.dram_tensor("iota", [P, P], BF16, kind="ExternalInput")
    out = nc.dram_tensor("out", [W, P, D], F32, kind="ExternalOutput")

    with tile.TileContext(nc) as tc, ExitStack() as ctx:
        const = ctx.enter_context(tc.tile_pool(name="const", bufs=1))
        hpool = ctx.enter_context(tc.tile_pool(name="hw", bufs=8))
        ohpool = ctx.enter_context(tc.tile_pool(name="oh", bufs=4))
        opool = ctx.enter_context(tc.tile_pool(name="ot", bufs=8))
        psum = ctx.enter_context(tc.tile_pool(name="acc", bufs=4, space="PSUM"))

        halves = [(0, T // 2), (T // 2, T)]
        hloads = halves if planes == 2 else [(0, T)]

        def load_h(w, t0, t1):
            ht = hpool.tile([P, (t1 - t0) * planes * D], hdt, tag="ht")
            nc.sync.dma_start(ht[:], h[w][:, t0 * planes * D : t1 * planes * D])
            return ht

        hts = {}
        for w in range(2):
            for t0, t1 in hloads:
                hts[(w, t0)] = load_h(w, t0, t1)

        iota_sb = const.tile([P, P], BF16)
        nc.scalar.dma_start(iota_sb[:], iota[:])
        lid_sb = const.tile([P, W * T], BF16)
        nc.scalar.dma_start(lid_sb[:], lid[:])

        for w in range(W):
            acc = psum.tile([P, planes * D], F32)
            for t0, t1 in hloads:
                if (w, t0) not in hts:
                    hts[(w, t0)] = load_h(w, t0, t1)
            for t0, t1 in halves:
                th = t1 - t0
                if planes == 2:
                    ht, hoff = hts[(w, t0)], 0
                else:
                    ht, hoff = hts[(w, 0)], t0 * D
                oh = ohpool.tile([P, th, P], hdt)
                nc.vector.tensor_tensor(
                    oh[:],
                    iota_sb[:].unsqueeze(1).broadcast_to((P, th, P)),
                    lid_sb[:, w * T + t0 : w * T + t1]
                    .unsqueeze(2)
                    .broadcast_to((P, th, P)),
                    mybir.AluOpType.is_equal,
                )
                for t in range(th):
                    nc.tensor.matmul(
                        acc[:],
                        oh[:, t, :],
                        ht[:, hoff + planes * t * D : hoff + planes * (t + 1) * D],
                        start=(t0 == 0 and t == 0),
                        stop=(t1 == T and t == th - 1),
                    )
            ot = opool.tile([P, D], F32)
            nc.scalar.copy(ot[:], acc[:, :D])
            if planes == 2:
                nc.vector.tensor_tensor(
                    ot[:], ot[:], acc[:, D:], mybir.AluOpType.add
                )
            nc.scalar.dma_start(out[w], ot[:])

    nc.compile()
    return nc


def _prepare_wide(H: np.ndarray, X: np.ndarray, V: int, mode: str):
    planes = 2 if mode == "bf16x2" else 1
    N, Dd = H.shape
    assert Dd == D and N % N_CORES == 0
    nloc = N // N_CORES
    X = np.ascontiguousarray(X).astype(np.int64, copy=False)
    perm, sidx, bounds, T, W = _sort_and_window(X, V, P, T_WIDE_CANDIDATES)
    cap = T * P

    wbase, win, rank = _row_windows(sidx, bounds, W, V, nloc)
    k_arr = np.repeat(np.arange(N_CORES), nloc)
    lid_val = sidx - wbase[k_arr, win]
    slot = (k_arr * W + win) * cap + (rank & (P - 1)) * T + (rank >> 7)

    total = N_CORES * W * cap

    if mode == "fp8w128":
        Qs = _quantize_fp8_feedback(H[perm], sidx, nloc)
        Hp = np.zeros((total, D), NP_FP8)
        Hp[slot] = Qs
        Hp = Hp.reshape(N_CORES, W, P, T * D)
    else:
        src = np.zeros(total, np.int64)
        src[slot] = perm
        hi = H.astype(NP_BF16)
        Hp = np.empty((total, planes, D), NP_BF16)
        Hp[:, 0, :] = hi[src]
        if planes == 2:
            lo = (H - hi.astype(np.float32)).astype(NP_BF16)
            Hp[:, 1, :] = lo[src]
        Hp = Hp.reshape(N_CORES, W, P, T * planes * D)

    lid = np.full(total, PAD_LID, NP_BF16)
    lid[slot] = lid_val.astype(NP_BF16)
    lid = (
        lid.reshape(N_CORES, W, P, T).transpose(0, 2, 1, 3).reshape(N_CORES, P, W * T)
    )
    lid = np.ascontiguousarray(lid)

    iota = np.ascontiguousarray(
        np.broadcast_to(np.arange(P, dtype=np.float32).astype(NP_BF16), (P, P))
    )

    in_maps = [{"h": Hp[k], "lid": lid[k], "iota": iota} for k in range(N_CORES)]
    return in_maps, wbase, W, T


def _kernel_wide(H, X, V, mode, trace):
    global LAST_RESULTS
    in_maps, wbase, W, T = _prepare_wide(H, X, V, mode)
    key = (mode, W, T)
    if key not in _NC_CACHE:
        _NC_CACHE[key] = _build_nc_wide(W, T, mode)
    nc = _NC_CACHE[key]
    res = run_bass_kernel_spmd(nc, in_maps, list(range(N_CORES)), trace=trace)
    LAST_RESULTS = res

    out = np.zeros((V + P, D), np.float32)
    for k in range(N_CORES):
        o = np.asarray(res.results[k]["out"])
        for w in range(W):
            b = int(wbase[k, w])
            out[b : b + P] += o[w]
    return np.ascontiguousarray(out[:V])


def kernel(H, X_node, V, trace: bool = False) -> np.ndarray:
    H = np.asarray(H, dtype=np.float32)
    X = np.asarray(X_node)
    V = int(V)
    mode = _mode()
    if mode == "fp8":
        return _kernel_narrow(H, X, V, trace)
    return _kernel_wide(H, X, V, mode, trace)
